# revision 30
# baseline (speedup 1.0000x reference)
"""AdditiveAttention Trainium2 kernel (8 NeuronCores, data-parallel over batch).

Reference computation (B=32, T=2048, D=U=512, fp32):
    query = values[:, -1] @ W2_w + W2_b                     # [B, U]
    keys  = values @ W1_w + W1_b                            # [B, T, U]
    score = tanh(keys + query[:, None, :]) @ V_w + V_b      # [B, T, 1]
    attn  = softmax(score, axis=1)
    out   = sum(attn * values, axis=1)                      # [B, D]

Sharding: data-parallel over B (4 batches per core), weights replicated,
no collectives.  bf16 matmuls (fp32 PSUM accumulate); rel-err ~3e-3.

Design notes (v3):
  - values pre-transposed on the HOST into valt [b, r, c, t] and
    pre-tiled natural nat [b, r, k, d]: every DMA is a plain copy with
    16KB-contiguous per-partition lines (no xbar transpose mode)
  - keys: per (b, u, half) 8 matmuls into kp [P, 2, TS], banks
    alternated (j inner) so consecutive matmuls hit different PSUM banks
  - score: 4 strips col-tiled at PSUM partitions 32s; the group for
    (b, u) is emitted AFTER the next keys half-block so the PE never
    stalls waiting for tanh (in-order engine)
  - softmax: Exp per strip with accum_out -> zpart[32s, b]; Z_b folds
    to partition 32b via a ones-stationary 1-col matmul; 1/Z lands at
    the same partition as the wsum strip so the final scale is one op
  - e rows assembled into e4 [4, T] by one SBUF->SBUF DMA per batch
  - tail: 16 e4-chunk transposes back-to-back, then 16 col-tiled wsum
    groups (4 batches concurrently, ~455ns per 4x512-col group)
"""

from contextlib import ExitStack

import numpy as np
import ml_dtypes

import concourse.bass as bass
import concourse.tile as tile
from concourse import bacc, mybir
from concourse.bass_utils import run_bass_kernel_spmd

BF16 = ml_dtypes.bfloat16

B, T, D, U = 32, 2048, 512, 512
NCORES = 8
BSH = B // NCORES          # 4 batches per core
P = 128
DC = D // P                # 4 chunks of D
UC = U // P                # 4 chunks of U
TS = 512                   # score strip / T tile
TN = T // TS               # 4 strips
TK = T // P                # 16 chunks of T for the weighted sum

_GRAPH = None


def _build_graph():
    nc = bacc.Bacc("TRN2", target_bir_lowering=False, debug=False)
    bf = mybir.dt.bfloat16
    f32 = mybir.dt.float32

    valt = nc.declare_dram_parameter("valt", [BSH, P, DC, T], bf, isOutput=False)
    valt0a = nc.declare_dram_parameter("valt0a", [P, DC, T // 2], bf, isOutput=False)
    valt0b = nc.declare_dram_parameter("valt0b", [P, DC, T // 2], bf, isOutput=False)
    nat = nc.declare_dram_parameter("nat", [BSH, P, TK, D], bf, isOutput=False)
    w1 = nc.declare_dram_parameter("w1", [P, DC, U], bf, isOutput=False)
    w2a = nc.declare_dram_parameter("w2a", [P, DC, P], bf, isOutput=False)
    w2b = nc.declare_dram_parameter("w2b", [P, UC - 1, DC, P], bf, isOutput=False)
    vw = nc.declare_dram_parameter("vw", [P, UC], bf, isOutput=False)
    bsum = nc.declare_dram_parameter("bsum", [P, UC], f32, isOutput=False)
    ones = nc.declare_dram_parameter("ones", [P, 1], f32, isOutput=False)
    lastrows = nc.declare_dram_parameter("lastrows", [BSH, D], bf, isOutput=False)
    ident = nc.declare_dram_parameter("ident", [BSH, BSH], bf, isOutput=False)
    out_ext = nc.declare_dram_parameter("out", [BSH, D], f32, isOutput=True)

    Tanh = mybir.ActivationFunctionType.Tanh
    Exp = mybir.ActivationFunctionType.Exp

    with tile.TileContext(nc) as tc, ExitStack() as ctx:
        const = ctx.enter_context(tc.tile_pool(name="const", bufs=1))
        valt_pool = ctx.enter_context(tc.tile_pool(name="valt", bufs=BSH))
        nat_pool = ctx.enter_context(tc.tile_pool(name="nat", bufs=BSH))
        tk_pool = ctx.enter_context(tc.tile_pool(name="tk", bufs=4))
        kps = ctx.enter_context(tc.tile_pool(name="kps", bufs=2, space="PSUM"))
        sps = ctx.enter_context(tc.tile_pool(name="sps", bufs=2, space="PSUM"))
        aps = ctx.enter_context(tc.tile_pool(name="aps", bufs=2, space="PSUM"))

        # ---- bulk loads: one sync FIFO, ordered so the query path (w2)
        # unblocks the in-order PE first, then w1 + the contiguous first-
        # batch halves feed keys as early as possible
        w1_sb = const.tile([P, DC, U], bf)
        nc.sync.dma_start(w1_sb[:], w1.ap())
        ident_sb = const.tile([BSH, BSH], bf)
        nc.sync.dma_start(ident_sb[:], ident.ap())
        lastrows_sb = const.tile([BSH, D], bf)
        nc.sync.dma_start(lastrows_sb[:], lastrows.ap())
        w2_sb = const.tile([P, UC, DC, P], bf)
        nc.sync.dma_start(w2_sb[:, 0], w2a.ap())
        bsum_sb = const.tile([P, UC], f32)
        nc.sync.dma_start(bsum_sb[:], bsum.ap())
        valt0_sb = valt_pool.tile([P, 2, DC, T // 2], bf, tag="valt")
        nc.sync.dma_start(valt0_sb[:, 0], valt0a.ap())
        v_sb = const.tile([P, UC], bf)
        nc.sync.dma_start(v_sb[:], vw.ap())
        ones_sb = const.tile([P, 1], f32)
        nc.sync.dma_start(ones_sb[:], ones.ap())
        nc.sync.dma_start(valt0_sb[:, 1], valt0b.ap())
        nc.sync.dma_start(w2_sb[:, 1:UC], w2b.ap())
        valts = [valt0_sb]
        for b in range(1, BSH):
            valt_b = valt_pool.tile([P, DC, T], bf, tag="valt")
            nc.sync.dma_start(valt_b[:], valt.ap()[b])
            valts.append(valt_b)
        nats = []
        for b in range(BSH):
            nat_b = nat_pool.tile([P, TK, D], bf, tag="nat")
            nc.sync.dma_start(nat_b[:], nat.ap()[b])
            nats.append(nat_b)

        # ---- query: lastrows -> lastT via PE transpose; q = lastT.T@W2
        lastT = const.tile([P, DC, BSH], bf)
        for c in range(DC):
            lp = aps.tile([P, BSH], bf, tag="aps")
            nc.tensor.transpose(
                lp[:], lastrows_sb[:, c * P : (c + 1) * P], ident_sb[:]
            )
            nc.vector.tensor_copy(lastT[:, c, :], lp[:])

        qb = const.tile([P, UC, BSH], f32)

        def emit_query(u):
            qp = aps.tile([P, BSH], f32, tag="aps")
            for c in range(DC):
                nc.tensor.matmul(
                    qp[:],
                    w2_sb[:, u, c, :],
                    lastT[:, c, :],
                    start=(c == 0),
                    stop=(c == DC - 1),
                )
            nc.vector.tensor_scalar_add(qb[:, u], qp[:], bsum_sb[:, u : u + 1])

        emit_query(0)

        # softmax state: e rows + per-strip Z partials + 1/Z at part 32b
        e_big = const.tile([P, BSH, TS], bf)
        e4a = const.tile([BSH - 1, T], bf)
        e4b = const.tile([1, T], bf)
        zpart = const.tile([P, BSH], f32)
        nc.scalar.memzero(zpart[:])
        zrec = const.tile([P, 1], f32)
        scps = []
        tkts = {}

        def emit_score(b, u):
            # 4 strips col-tiled across PE column groups; emitted one
            # keys half-block late so tanh(h1) is already done
            for s in range(TN):
                nc.tensor.matmul(
                    scps[b][32 * s : 32 * s + 1, :],
                    v_sb[:, u : u + 1],
                    tkts[(u, s // 2)][:, s % 2, :],
                    start=(u == 0),
                    stop=(u == UC - 1),
                    tile_position=(0, 32 * s),
                    skip_group_check=True,
                )

        def emit_exp(b):
            # one Exp over the whole bank: off-strip rows hold stale but
            # bounded scores, are never read, and the Z mask zeroes them
            nc.scalar.activation(
                e_big[:, b, :],
                scps[b][:, :],
                Exp,
                accum_out=zpart[:, b : b + 1],
            )
            # assemble row b of e4 on the scalar queue (same engine as
            # the Exp that produced it; sync stays pure bulk loads)
            q = nc.sync if b == BSH - 1 else nc.scalar
            dst = e4b[0:1, :] if b == BSH - 1 else e4a[b : b + 1, :]
            q.dma_start(
                dst.rearrange("p (s x) -> p s x", s=TN),
                e_big[0 : 3 * 32 + 1 : 32, b, :],
            )

        def emit_z(b):
            # Z_b = mask . zpart[:, b] -> partition 32b (mask keeps only
            # the 4 strip rows); deferred so the PE never waits on Exp
            zq = aps.tile([P, 1], f32, tag="aps")
            nc.tensor.matmul(
                zq[32 * b : 32 * b + 1, :],
                ones_sb[:],
                zpart[:, b : b + 1],
                start=True,
                stop=True,
                tile_position=(0, 32 * b),
                skip_group_check=True,
            )
            nc.vector.reciprocal(
                zrec[32 * b : 32 * b + 1, :], zq[32 * b : 32 * b + 1, :]
            )

        def emit_keys_half(b, u, h):
            kp = kps.tile([P, 2, TS], f32, tag="kps")
            for c in range(DC):
                for j in range(2):
                    mov = (
                        valts[0][:, h, c, j * TS : (j + 1) * TS]
                        if b == 0
                        else valts[b][:, c, (2 * h + j) * TS : (2 * h + j + 1) * TS]
                    )
                    nc.tensor.matmul(
                        kp[:, j],
                        w1_sb[:, c, u * P : (u + 1) * P],
                        mov,
                        start=(c == 0),
                        stop=(c == DC - 1),
                    )
            tkt = tk_pool.tile([P, 2, TS], bf, tag="tk")
            nc.scalar.activation(tkt[:], kp[:], Tanh, bias=qb[:, u, b : b + 1])
            tkts[(u, h)] = tkt

        # ---- main phase: keys -> tanh -> (delayed) score, batch-major --
        pending = None          # (b, u) whose score group is not yet out
        for b in range(BSH):
            scp = sps.tile([P, TS], f32, tag="sps")
            # zero the whole bank: the full-bank Exp reads every row, and
            # exp(stale PSUM) can be Inf, which would NaN the masked Z sum
            nc.vector.memset(scp[:], 0.0)
            scps.append(scp)
            for u in range(UC):
                emit_keys_half(b, u, 0)
                if pending is not None:
                    emit_score(*pending)
                    if pending[1] == UC - 1:
                        emit_exp(pending[0])
                    pending = None
                if b == 0 and u + 1 < UC:
                    emit_query(u + 1)
                if u == 1 and b > 0:
                    emit_z(b - 1)
                emit_keys_half(b, u, 1)
                pending = (b, u)
        emit_score(*pending)
        emit_exp(pending[0])

        # ---- tail: all e4-chunk transposes, then col-tiled wsum --------
        wp_a = sps.tile([P, D], f32, tag="sps")   # strips b0 (p0), b1 (p32)
        wp_b = sps.tile([P, D], f32, tag="sps")   # strips b2 (p64), b3 (p96)
        # b0-2 transposes depend only on e4a (landed mid-main), so they
        # fill the PE-idle window while exp(b3)/e4b-DMA latency elapses
        at_sb = const.tile([P, TK, BSH - 1], bf)
        for k in range(TK):
            at_p = aps.tile([P, BSH - 1], bf, tag="aps")
            nc.tensor.transpose(
                at_p[:], e4a[:, k * P : (k + 1) * P],
                ident_sb[0 : BSH - 1, 0 : BSH - 1],
            )
            if k % 2 == 0:
                nc.vector.tensor_copy(at_sb[:, k, :], at_p[:])
            else:
                nc.scalar.copy(at_sb[:, k, :], at_p[:])
        emit_z(BSH - 1)
        # b3 columns: legal base-0 single-row transposes of e4b, pipelined
        # forward-k with the wsum groups
        at3 = const.tile([P, TK], bf)
        for k in range(TK):
            a3p = aps.tile([P, 1], bf, tag="aps")
            nc.tensor.transpose(
                a3p[:], e4b[0:1, k * P : (k + 1) * P], ident_sb[0:1, 0:1]
            )
            if k % 2 == 0:
                nc.vector.tensor_copy(at3[:, k : k + 1], a3p[:])
            else:
                nc.scalar.copy(at3[:, k : k + 1], a3p[:])
            for b in range(BSH):
                wp = wp_a if b < 2 else wp_b
                at_col = (
                    at3[:, k : k + 1]
                    if b == BSH - 1
                    else at_sb[:, k, b : b + 1]
                )
                nc.tensor.matmul(
                    wp[32 * b : 32 * b + 1, :],
                    at_col,
                    nats[b][:, k, :],
                    start=(k == 0),
                    stop=(k == TK - 1),
                    tile_position=(0, 32 * b),
                    skip_group_check=True,
                )

        # out rows = wp strip * (1/Z); two PSUM banks + engine-bank
        # pairing so ACT and DVE never read the same bank at once
        ob_a = const.tile([P, D], f32)
        ob_v = const.tile([P, D], f32)
        nc.scalar.mul(ob_a[0:1, :], wp_a[0:1, :], zrec[0:1, 0:1])
        nc.vector.tensor_scalar_mul(
            ob_v[64:65, :], wp_b[64:65, :], zrec[64:65, 0:1]
        )
        nc.scalar.mul(ob_a[96:97, :], wp_b[96:97, :], zrec[96:97, 0:1])
        nc.vector.tensor_scalar_mul(
            ob_v[32:33, :], wp_a[32:33, :], zrec[32:33, 0:1]
        )
        nc.sync.dma_start(out_ext.ap()[0:4:3, :], ob_a[0:97:96, :])
        nc.scalar.dma_start(out_ext.ap()[1:3, :], ob_v[32:65:32, :])

    nc.finalize()
    return nc


def _get_graph():
    global _GRAPH
    if _GRAPH is None:
        _GRAPH = _build_graph()
    return _GRAPH


def _make_in_maps(values, W1_w, W1_b, W2_w, W2_b, V_w, V_b):
    vals = np.asarray(values, np.float32)
    w1_bf = np.ascontiguousarray(
        np.asarray(W1_w, np.float32).reshape(DC, P, U).transpose(1, 0, 2)
    ).astype(BF16)
    w2_um = (
        np.asarray(W2_w, np.float32)
        .reshape(DC, P, UC, P)
        .transpose(1, 2, 0, 3)
    )  # [r, u, c, m]
    w2a_bf = np.ascontiguousarray(w2_um[:, 0]).astype(BF16)
    w2b_bf = np.ascontiguousarray(w2_um[:, 1:]).astype(BF16)
    v_bf = np.ascontiguousarray(
        np.asarray(V_w, np.float32).reshape(UC, P).T
    ).astype(BF16)
    bsum = np.ascontiguousarray(
        (np.asarray(W1_b, np.float32) + np.asarray(W2_b, np.float32))
        .reshape(UC, P)
        .T
    )
    ones = np.zeros((P, 1), dtype=np.float32)
    ones[0 : 3 * 32 + 1 : 32] = 1.0
    ident = np.eye(BSH, dtype=BF16)

    in_maps = []
    for core in range(NCORES):
        sl = vals[core * BSH : (core + 1) * BSH]  # [BSH, T, D] f32
        # valt[b, r, c, t] = v[b, t, 128c + r]
        valt = np.ascontiguousarray(
            sl.reshape(BSH, T, DC, P).transpose(0, 3, 2, 1)
        ).astype(BF16)
        # nat[b, r, k, d] = v[b, 128k + r, d]
        nat = np.ascontiguousarray(
            sl.reshape(BSH, TK, P, D).transpose(0, 2, 1, 3)
        ).astype(BF16)
        lastrows = np.ascontiguousarray(sl[:, T - 1, :]).astype(BF16)
        in_maps.append(
            {
                "valt": valt,
                "valt0a": np.ascontiguousarray(valt[0][:, :, 0 : T // 2]),
                "valt0b": np.ascontiguousarray(valt[0][:, :, T // 2 : T]),
                "nat": nat,
                "w1": w1_bf,
                "w2a": w2a_bf,
                "w2b": w2b_bf,
                "vw": v_bf,
                "bsum": bsum,
                "ones": ones,
                "lastrows": lastrows,
                "ident": ident,
            }
        )
    return in_maps


def run(inputs, trace=False, **kw):
    """Build + run on 8 cores; returns (full_output, BassKernelResults)."""
    nc = _get_graph()
    in_maps = _make_in_maps(**inputs)
    res = run_bass_kernel_spmd(
        nc, in_maps, core_ids=list(range(NCORES)), trace=trace, **kw
    )
    out = np.concatenate([np.asarray(r["out"]) for r in res.results], axis=0)
    return out.astype(np.float32), res


def kernel(**inputs) -> np.ndarray:
    out, _ = run(inputs)
    return out


# revision 31
# speedup vs baseline: 1.0488x; 1.0488x over previous
"""AdditiveAttention Trainium2 kernel (8 NeuronCores, data-parallel over batch).

Reference computation (B=32, T=2048, D=U=512, fp32):
    query = values[:, -1] @ W2_w + W2_b                     # [B, U]
    keys  = values @ W1_w + W1_b                            # [B, T, U]
    score = tanh(keys + query[:, None, :]) @ V_w + V_b      # [B, T, 1]
    attn  = softmax(score, axis=1)
    out   = sum(attn * values, axis=1)                      # [B, D]

Sharding: data-parallel over B (4 batches per core), weights replicated,
no collectives.  bf16 matmuls (fp32 PSUM accumulate); rel-err ~3e-3.

Design notes (v3):
  - values pre-transposed on the HOST into valt [b, r, c, t] and
    pre-tiled natural nat [b, r, k, d]: every DMA is a plain copy with
    16KB-contiguous per-partition lines (no xbar transpose mode)
  - keys: per (b, u, half) 8 matmuls into kp [P, 2, TS], banks
    alternated (j inner) so consecutive matmuls hit different PSUM banks
  - score: 4 strips col-tiled at PSUM partitions 32s; the group for
    (b, u) is emitted AFTER the next keys half-block so the PE never
    stalls waiting for tanh (in-order engine)
  - softmax: Exp per strip with accum_out -> zpart[32s, b]; Z_b folds
    to partition 32b via a ones-stationary 1-col matmul; 1/Z lands at
    the same partition as the wsum strip so the final scale is one op
  - e rows assembled into e4 [4, T] by one SBUF->SBUF DMA per batch
  - tail: 16 e4-chunk transposes back-to-back, then 16 col-tiled wsum
    groups (4 batches concurrently, ~455ns per 4x512-col group)
"""

from contextlib import ExitStack

import numpy as np
import ml_dtypes

import concourse.bass as bass
import concourse.tile as tile
from concourse import bacc, mybir
from concourse.bass_utils import run_bass_kernel_spmd

BF16 = ml_dtypes.bfloat16

B, T, D, U = 32, 2048, 512, 512
NCORES = 8
BSH = B // NCORES          # 4 batches per core
P = 128
DC = D // P                # 4 chunks of D
UC = U // P                # 4 chunks of U
TS = 512                   # score strip / T tile
TN = T // TS               # 4 strips
TK = T // P                # 16 chunks of T for the weighted sum

_GRAPH = None


def _build_graph():
    nc = bacc.Bacc("TRN2", target_bir_lowering=False, debug=False)
    bf = mybir.dt.bfloat16
    f32 = mybir.dt.float32

    valt = nc.declare_dram_parameter("valt", [BSH, P, DC, T], bf, isOutput=False)
    valt0a = nc.declare_dram_parameter("valt0a", [P, DC, T // 2], bf, isOutput=False)
    valt0b = nc.declare_dram_parameter("valt0b", [P, DC, T // 2], bf, isOutput=False)
    nat = nc.declare_dram_parameter("nat", [BSH, P, TK, D], bf, isOutput=False)
    w1 = nc.declare_dram_parameter("w1", [P, DC, U], bf, isOutput=False)
    w2a = nc.declare_dram_parameter("w2a", [P, DC, P], bf, isOutput=False)
    w2b = nc.declare_dram_parameter("w2b", [P, UC - 1, DC, P], bf, isOutput=False)
    vw = nc.declare_dram_parameter("vw", [P, UC], bf, isOutput=False)
    bsum = nc.declare_dram_parameter("bsum", [P, UC], f32, isOutput=False)
    ones = nc.declare_dram_parameter("ones", [P, 1], f32, isOutput=False)
    lastrows = nc.declare_dram_parameter("lastrows", [BSH, D], bf, isOutput=False)
    ident = nc.declare_dram_parameter("ident", [BSH, BSH], bf, isOutput=False)
    out_ext = nc.declare_dram_parameter("out", [BSH, D], f32, isOutput=True)

    Tanh = mybir.ActivationFunctionType.Tanh
    Exp = mybir.ActivationFunctionType.Exp

    with tile.TileContext(nc) as tc, ExitStack() as ctx:
        const = ctx.enter_context(tc.tile_pool(name="const", bufs=1))
        valt_pool = ctx.enter_context(tc.tile_pool(name="valt", bufs=BSH))
        nat_pool = ctx.enter_context(tc.tile_pool(name="nat", bufs=BSH))
        tk_pool = ctx.enter_context(tc.tile_pool(name="tk", bufs=4))
        kps = ctx.enter_context(tc.tile_pool(name="kps", bufs=2, space="PSUM"))
        sps = ctx.enter_context(tc.tile_pool(name="sps", bufs=2, space="PSUM"))
        aps = ctx.enter_context(tc.tile_pool(name="aps", bufs=2, space="PSUM"))

        # ---- bulk loads: one sync FIFO, ordered so the query path (w2)
        # unblocks the in-order PE first, then w1 + the contiguous first-
        # batch halves feed keys as early as possible
        w1_sb = const.tile([P, DC, U], bf)
        nc.sync.dma_start(w1_sb[:], w1.ap())
        ident_sb = const.tile([BSH, BSH], bf)
        nc.sync.dma_start(ident_sb[:], ident.ap())
        lastrows_sb = const.tile([BSH, D], bf)
        nc.sync.dma_start(lastrows_sb[:], lastrows.ap())
        w2_sb = const.tile([P, UC, DC, P], bf)
        nc.sync.dma_start(w2_sb[:, 0], w2a.ap())
        bsum_sb = const.tile([P, UC], f32)
        nc.sync.dma_start(bsum_sb[:], bsum.ap())
        valt0_sb = valt_pool.tile([P, 2, DC, T // 2], bf, tag="valt")
        nc.sync.dma_start(valt0_sb[:, 0], valt0a.ap())
        v_sb = const.tile([P, UC], bf)
        nc.sync.dma_start(v_sb[:], vw.ap())
        ones_sb = const.tile([P, 1], f32)
        nc.sync.dma_start(ones_sb[:], ones.ap())
        nc.sync.dma_start(valt0_sb[:, 1], valt0b.ap())
        nc.sync.dma_start(w2_sb[:, 1:UC], w2b.ap())
        valts = [valt0_sb]
        for b in range(1, BSH):
            valt_b = valt_pool.tile([P, DC, T], bf, tag="valt")
            nc.sync.dma_start(valt_b[:], valt.ap()[b])
            valts.append(valt_b)
        nats = []
        for b in range(BSH):
            nat_b = nat_pool.tile([P, TK, D], bf, tag="nat")
            nc.sync.dma_start(nat_b[:], nat.ap()[b])
            nats.append(nat_b)

        # ---- query: lastrows -> lastT via PE transpose; q = lastT.T@W2
        lastT = const.tile([P, DC, BSH], bf)
        for c in range(DC):
            lp = aps.tile([P, BSH], bf, tag="aps")
            nc.tensor.transpose(
                lp[:], lastrows_sb[:, c * P : (c + 1) * P], ident_sb[:]
            )
            nc.vector.tensor_copy(lastT[:, c, :], lp[:])

        qb = const.tile([P, UC, BSH], f32)

        def emit_query(u):
            qp = aps.tile([P, BSH], f32, tag="aps")
            for c in range(DC):
                nc.tensor.matmul(
                    qp[:],
                    w2_sb[:, u, c, :],
                    lastT[:, c, :],
                    start=(c == 0),
                    stop=(c == DC - 1),
                )
            nc.vector.tensor_scalar_add(qb[:, u], qp[:], bsum_sb[:, u : u + 1])

        emit_query(0)

        # softmax state: e rows + per-strip Z partials + 1/Z at part 32b
        e_big = const.tile([P, BSH, TS], bf)
        e4 = const.tile([BSH, T], bf)
        zpart = const.tile([P, BSH], f32)
        nc.scalar.memzero(zpart[:])
        zrec = const.tile([P, 1], f32)
        scps = []
        tkts = {}

        def emit_score(b, u):
            # 4 strips col-tiled across PE column groups; emitted one
            # keys half-block late so tanh(h1) is already done
            for s in range(TN):
                nc.tensor.matmul(
                    scps[b][32 * s : 32 * s + 1, :],
                    v_sb[:, u : u + 1],
                    tkts[(u, s // 2)][:, s % 2, :],
                    start=(u == 0),
                    stop=(u == UC - 1),
                    tile_position=(0, 32 * s),
                    skip_group_check=True,
                )

        def emit_exp(b):
            # one Exp over the whole bank: off-strip rows hold stale but
            # bounded scores, are never read, and the Z mask zeroes them
            nc.scalar.activation(
                e_big[:, b, :],
                scps[b][:, :],
                Exp,
                accum_out=zpart[:, b : b + 1],
            )
            # assemble row b of e4 on the scalar queue (same engine as
            # the Exp that produced it; sync stays pure bulk loads)
            q = nc.sync if b == BSH - 1 else nc.scalar
            q.dma_start(
                e4[b : b + 1, :].rearrange("p (s x) -> p s x", s=TN),
                e_big[0 : 3 * 32 + 1 : 32, b, :],
            )

        def emit_z(b):
            # Z_b = mask . zpart[:, b] -> partition 32b (mask keeps only
            # the 4 strip rows); deferred so the PE never waits on Exp
            zq = aps.tile([P, 1], f32, tag="aps")
            nc.tensor.matmul(
                zq[32 * b : 32 * b + 1, :],
                ones_sb[:],
                zpart[:, b : b + 1],
                start=True,
                stop=True,
                tile_position=(0, 32 * b),
                skip_group_check=True,
            )
            nc.vector.reciprocal(
                zrec[32 * b : 32 * b + 1, :], zq[32 * b : 32 * b + 1, :]
            )

        def emit_keys_half(b, u, h):
            kp = kps.tile([P, 2, TS], f32, tag="kps")
            for c in range(DC):
                for j in range(2):
                    mov = (
                        valts[0][:, h, c, j * TS : (j + 1) * TS]
                        if b == 0
                        else valts[b][:, c, (2 * h + j) * TS : (2 * h + j + 1) * TS]
                    )
                    nc.tensor.matmul(
                        kp[:, j],
                        w1_sb[:, c, u * P : (u + 1) * P],
                        mov,
                        start=(c == 0),
                        stop=(c == DC - 1),
                    )
            tkt = tk_pool.tile([P, 2, TS], bf, tag="tk")
            nc.scalar.activation(tkt[:], kp[:], Tanh, bias=qb[:, u, b : b + 1])
            tkts[(u, h)] = tkt

        # ---- main phase: keys -> tanh -> (delayed) score, batch-major --
        pending = None          # (b, u) whose score group is not yet out
        for b in range(BSH):
            scp = sps.tile([P, TS], f32, tag="sps")
            # zero the whole bank: the full-bank Exp reads every row, and
            # exp(stale PSUM) can be Inf, which would NaN the masked Z sum
            nc.vector.memset(scp[:], 0.0)
            scps.append(scp)
            for u in range(UC):
                emit_keys_half(b, u, 0)
                if pending is not None:
                    emit_score(*pending)
                    if pending[1] == UC - 1:
                        emit_exp(pending[0])
                    pending = None
                if b == 0 and u + 1 < UC:
                    emit_query(u + 1)
                if u == 1 and b > 0:
                    emit_z(b - 1)
                emit_keys_half(b, u, 1)
                pending = (b, u)
        emit_score(*pending)
        emit_exp(pending[0])
        emit_z(BSH - 1)

        # ---- tail: all e4-chunk transposes, then col-tiled wsum --------
        wp_a = sps.tile([P, D], f32, tag="sps")   # strips b0 (p0), b1 (p32)
        wp_b = sps.tile([P, D], f32, tag="sps")   # strips b2 (p64), b3 (p96)
        at_sb = const.tile([P, TK, BSH], bf)
        at_ps = []
        for k in range(TK):
            at_p = aps.tile([P, BSH], bf, tag="aps")
            nc.tensor.transpose(
                at_p[:], e4[:, k * P : (k + 1) * P], ident_sb[:]
            )
            if k % 2 == 0:
                nc.vector.tensor_copy(at_sb[:, k, :], at_p[:])
            else:
                nc.scalar.copy(at_sb[:, k, :], at_p[:])
        for k in range(TK - 1, -1, -1):
            for b in range(BSH):
                wp = wp_a if b < 2 else wp_b
                nc.tensor.matmul(
                    wp[32 * b : 32 * b + 1, :],
                    at_sb[:, k, b : b + 1],
                    nats[b][:, k, :],
                    start=(k == TK - 1),
                    stop=(k == 0),
                    tile_position=(0, 32 * b),
                    skip_group_check=True,
                )

        # out rows = wp strip * (1/Z); two PSUM banks + engine-bank
        # pairing so ACT and DVE never read the same bank at once
        ob_a = const.tile([P, D], f32)
        ob_v = const.tile([P, D], f32)
        nc.scalar.mul(ob_a[0:1, :], wp_a[0:1, :], zrec[0:1, 0:1])
        nc.vector.tensor_scalar_mul(
            ob_v[64:65, :], wp_b[64:65, :], zrec[64:65, 0:1]
        )
        nc.scalar.mul(ob_a[96:97, :], wp_b[96:97, :], zrec[96:97, 0:1])
        nc.vector.tensor_scalar_mul(
            ob_v[32:33, :], wp_a[32:33, :], zrec[32:33, 0:1]
        )
        nc.sync.dma_start(out_ext.ap()[0:4:3, :], ob_a[0:97:96, :])
        nc.scalar.dma_start(out_ext.ap()[1:3, :], ob_v[32:65:32, :])

    nc.finalize()
    return nc


def _get_graph():
    global _GRAPH
    if _GRAPH is None:
        _GRAPH = _build_graph()
    return _GRAPH


def _make_in_maps(values, W1_w, W1_b, W2_w, W2_b, V_w, V_b):
    vals = np.asarray(values, np.float32)
    w1_bf = np.ascontiguousarray(
        np.asarray(W1_w, np.float32).reshape(DC, P, U).transpose(1, 0, 2)
    ).astype(BF16)
    w2_um = (
        np.asarray(W2_w, np.float32)
        .reshape(DC, P, UC, P)
        .transpose(1, 2, 0, 3)
    )  # [r, u, c, m]
    w2a_bf = np.ascontiguousarray(w2_um[:, 0]).astype(BF16)
    w2b_bf = np.ascontiguousarray(w2_um[:, 1:]).astype(BF16)
    v_bf = np.ascontiguousarray(
        np.asarray(V_w, np.float32).reshape(UC, P).T
    ).astype(BF16)
    bsum = np.ascontiguousarray(
        (np.asarray(W1_b, np.float32) + np.asarray(W2_b, np.float32))
        .reshape(UC, P)
        .T
    )
    ones = np.zeros((P, 1), dtype=np.float32)
    ones[0 : 3 * 32 + 1 : 32] = 1.0
    ident = np.eye(BSH, dtype=BF16)

    in_maps = []
    for core in range(NCORES):
        sl = vals[core * BSH : (core + 1) * BSH]  # [BSH, T, D] f32
        # valt[b, r, c, t] = v[b, t, 128c + r]
        valt = np.ascontiguousarray(
            sl.reshape(BSH, T, DC, P).transpose(0, 3, 2, 1)
        ).astype(BF16)
        # nat[b, r, k, d] = v[b, 128k + r, d]
        nat = np.ascontiguousarray(
            sl.reshape(BSH, TK, P, D).transpose(0, 2, 1, 3)
        ).astype(BF16)
        lastrows = np.ascontiguousarray(sl[:, T - 1, :]).astype(BF16)
        in_maps.append(
            {
                "valt": valt,
                "valt0a": np.ascontiguousarray(valt[0][:, :, 0 : T // 2]),
                "valt0b": np.ascontiguousarray(valt[0][:, :, T // 2 : T]),
                "nat": nat,
                "w1": w1_bf,
                "w2a": w2a_bf,
                "w2b": w2b_bf,
                "vw": v_bf,
                "bsum": bsum,
                "ones": ones,
                "lastrows": lastrows,
                "ident": ident,
            }
        )
    return in_maps


def run(inputs, trace=False, **kw):
    """Build + run on 8 cores; returns (full_output, BassKernelResults)."""
    nc = _get_graph()
    in_maps = _make_in_maps(**inputs)
    res = run_bass_kernel_spmd(
        nc, in_maps, core_ids=list(range(NCORES)), trace=trace, **kw
    )
    out = np.concatenate([np.asarray(r["out"]) for r in res.results], axis=0)
    return out.astype(np.float32), res


def kernel(**inputs) -> np.ndarray:
    out, _ = run(inputs)
    return out
